# revision 1
# baseline (speedup 1.0000x reference)
"""Multi-head self-attention on 8 TRN2 NeuronCores — v3.

Same math/sharding as v1/v2 ((batch x query-half) shards, fp16 compute,
transposed-scores softmax with ones-column denominators), but restructured
for this platform's cost profile (measured: ~50us per matmul instruction,
~10us per DVE op, ACT ~free): matmul count minimized via N=1024 moving
operands, all inputs shipped in exact SBUF layout and loaded with one
contiguous DMA each.
"""

import os
import numpy as np

B, S, D = 4, 2048, 1024
H, DK = 16, 64
SQ = S // 2
FV = H * 65          # V' columns incl. per-head ones column
SCALE = 64 ** -0.5
NCORES = 8

_cache = {}
LAST_EXEC_TIME_NS = None

MMN = int(os.environ.get("KERNEL_MMN", "512"))   # moving free dim per matmul


def _build_nc(repeat=1):
    import concourse.bass as bass
    import concourse.mybir as mybir
    import concourse.tile as tile
    from concourse import bacc

    fp16 = mybir.dt.float16
    f32 = mybir.dt.float32
    mult = mybir.AluOpType.mult
    add = mybir.AluOpType.add

    nc = bacc.Bacc(target_bir_lowering=False, debug=False, num_devices=NCORES)

    # ---- DRAM parameters, already in SBUF layout ----
    xt_d = nc.dram_tensor("xt", [128, 8, S], fp16, kind="ExternalInput")
    xq_d = nc.dram_tensor("xq", [128, 8, SQ], fp16, kind="ExternalInput")
    wq_d = nc.dram_tensor("wq", [128, 64, 128], fp16, kind="ExternalInput")
    wk_d = nc.dram_tensor("wk", [128, 64, 128], fp16, kind="ExternalInput")
    wv_d = nc.dram_tensor("wv", [128, 8, 1024], fp16, kind="ExternalInput")  # dense V wT
    bq_d = nc.dram_tensor("bq", [128, 8], f32, kind="ExternalInput")
    bk_d = nc.dram_tensor("bk", [128, 8], f32, kind="ExternalInput")
    bv_d = nc.dram_tensor("bv", [1024], fp16, kind="ExternalInput")          # dense V bias
    pw_d = nc.dram_tensor("pw", [128, 8, 1024], fp16, kind="ExternalInput")
    pb_d = nc.dram_tensor("pb", [1024], f32, kind="ExternalInput")
    out_d = nc.dram_tensor("out", [SQ, D], f32, kind="ExternalOutput")

    def bcast_rows(ap, parts):
        return bass.AP(tensor=ap.tensor, offset=ap.offset, ap=[[0, parts], *ap.ap])

    def mm_chunks(total):
        c = []
        o = 0
        while o < total:
            n = min(MMN, total - o)
            c.append((o, n))
            o += n
        return c

    with tile.TileContext(nc) as tc:
        with (
            tc.tile_pool(name="const", bufs=1) as const,
            tc.tile_pool(name="xpool", bufs=1) as xpool,
            tc.tile_pool(name="acts", bufs=1) as acts,
            tc.tile_pool(name="qk", bufs=2) as qkpool,
            tc.tile_pool(name="estream", bufs=4) as estream,
            tc.tile_pool(name="small", bufs=3) as small,
            tc.tile_pool(name="ps", bufs=2, space="PSUM") as ps,
            tc.tile_pool(name="psO", bufs=2, space="PSUM") as psO,
            tc.tile_pool(name="dscr", bufs=2, space="DRAM") as dscr,
        ):
            bvb = const.tile([128, 1024], fp16, tag="bvb")
            nc.sync.dma_start(out=bvb, in_=bcast_rows(bv_d.ap(), 128))
            pbb = const.tile([128, 1024], f32, tag="pbb")
            nc.sync.dma_start(out=pbb, in_=bcast_rows(pb_d.ap(), 128))
            wq_all = const.tile([128, 64, 128], fp16, tag="wq_all")
            nc.sync.dma_start(out=wq_all, in_=wq_d.ap())
            wk_all = const.tile([128, 64, 128], fp16, tag="wk_all")
            nc.sync.dma_start(out=wk_all, in_=wk_d.ap())
            bq_all = const.tile([128, 8], f32, tag="bq_all")
            nc.sync.dma_start(out=bq_all, in_=bq_d.ap())
            bk_all = const.tile([128, 8], f32, tag="bk_all")
            nc.sync.dma_start(out=bk_all, in_=bk_d.ap())

            def body():
                xt = xpool.tile([128, 8, S], fp16, tag="xt", name="xt")
                nc.sync.dma_start(out=xt, in_=xt_d.ap())
                xq = xpool.tile([128, 8, SQ], fp16, tag="xq", name="xq")
                nc.sync.dma_start(out=xq, in_=xq_d.ap())
                # pw later reuses wv's slot (same tag) once V' is done
                wv = xpool.tile([128, 8, 1024], fp16, tag="wv", name="wv", bufs=1)
                nc.sync.dma_start(out=wv, in_=wv_d.ap())

                # ---- V' ----
                vt = []
                for st in range(16):
                    psa = ps.tile([128, 1024], f32, tag="ps", name="psa")
                    for dt in range(8):
                        for o, n in mm_chunks(1024):
                            nc.tensor.matmul(psa[:, o:o + n],
                                             xt[:, dt, st * 128:(st + 1) * 128],
                                             wv[:, dt, o:o + n],
                                             start=(dt == 0), stop=(dt == 7))
                    v = acts.tile([128, 16, 65], fp16, tag=f"v{st}", name=f"v{st}")
                    # dense [128,1024] psum + bias -> strided 64-col blocks of v
                    nc.vector.tensor_tensor(
                        v[:, :, 0:64],
                        psa.rearrange("p (a b) -> p a b", a=16),
                        bvb.rearrange("p (a b) -> p a b", a=16), add)
                    nc.vector.memset(v[:, :, 64], 1.0)
                    vt.append(v)

                otn = [acts.tile([128, SQ], fp16, tag=f"otn{i}", name=f"otn{i}")
                       for i in range(8)]

                def qk_pair(hp):
                    psq = ps.tile([128, SQ], f32, tag="ps", name="psq")
                    for dt in range(8):
                        for o, n in mm_chunks(SQ):
                            nc.tensor.matmul(psq[:, o:o + n],
                                             wq_all[:, hp * 8 + dt, :],
                                             xq[:, dt, o:o + n],
                                             start=(dt == 0), stop=(dt == 7))
                    qt = qkpool.tile([128, SQ], fp16, tag="qt", name="qt")
                    nc.vector.tensor_scalar(qt[:], psq, bq_all[:, hp:hp + 1], None, add)

                    kt_t = qkpool.tile([128, S], fp16, tag="kt", name="kt_t")
                    # dt outer: each wk stationary serves both sequence halves
                    # (4 matmuls) before swapping; both psum tiles accumulate
                    # in parallel across the dt loop (2 ps slots)
                    psk2 = [ps.tile([128, SQ], f32, tag="ps", name=f"psk{h}")
                            for h in range(2)]
                    for dt in range(8):
                        for half in range(2):
                            for o, n in mm_chunks(SQ):
                                nc.tensor.matmul(psk2[half][:, o:o + n],
                                                 wk_all[:, hp * 8 + dt, :],
                                                 xt[:, dt, half * SQ + o:half * SQ + o + n],
                                                 start=(dt == 0), stop=(dt == 7))
                    for half in range(2):
                        nc.vector.tensor_scalar(kt_t[:, half * SQ:(half + 1) * SQ],
                                                psk2[half], bk_all[:, hp:hp + 1], None, add)
                    return qt, kt_t

                def attention(hp, qt, kt_t):
                    ot2 = []
                    for hh in range(2):
                        ot = psO.tile([65, SQ], f32, tag="ot", name=f"ot{hh}")
                        ot2.append(ot)
                    for kt in range(16):
                        sc2 = []
                        for hh in range(2):
                            sc = ps.tile([128, SQ], f32, tag="ps", name=f"sc{hh}")
                            sc2.append(sc)
                        # chunks inner per head: consecutive matmuls share the
                        # stationary (KT slice) -> half the stationary swaps
                        for hh in range(2):
                            hsl = slice(hh * 64, (hh + 1) * 64)
                            for o, n in mm_chunks(SQ):
                                nc.tensor.matmul(
                                    sc2[hh][:, o:o + n],
                                    kt_t[hsl, kt * 128:(kt + 1) * 128],
                                    qt[hsl, o:o + n],
                                    start=True, stop=True)
                        for hh in range(2):
                            h = 2 * hp + hh
                            e = estream.tile([128, SQ], fp16, tag="e", name="e")
                            nc.scalar.activation(e[:], sc2[hh][:],
                                                 mybir.ActivationFunctionType.Exp,
                                                 scale=float(SCALE))
                            for o, n in mm_chunks(SQ):
                                nc.tensor.matmul(
                                    ot2[hh][:, o:o + n],
                                    vt[kt][:, h, :],
                                    e[:, o:o + n],
                                    start=(kt == 0), stop=(kt == 15))
                    for hh in range(2):
                        ot = ot2[hh]
                        rec = small.tile([1, SQ], f32, tag="rec", name="rec")
                        nc.vector.reciprocal(rec, ot[64:65, :])
                        recb = small.tile([64, SQ], f32, tag="recb", name="recb")
                        nc.gpsimd.partition_broadcast(recb, rec)
                        nc.vector.tensor_tensor(otn[hp][hh * 64:(hh + 1) * 64, :],
                                                ot[0:64, :], recb, mult)

                pend = qk_pair(0)
                for hp in range(8):
                    nxt = qk_pair(hp + 1) if hp < 7 else None
                    attention(hp, *pend)
                    pend = nxt

                # ---- output projection ----
                pw = xpool.tile([128, 8, 1024], fp16, tag="wv", name="pw", bufs=1)
                nc.sync.dma_start(out=pw, in_=pw_d.ap())
                for st in range(8):
                    pso = ps.tile([128, 1024], f32, tag="ps", name="pso")
                    for ft in range(8):
                        for o, n in mm_chunks(1024):
                            nc.tensor.matmul(pso[:, o:o + n],
                                             otn[ft][:, st * 128:(st + 1) * 128],
                                             pw[:, ft, o:o + n],
                                             start=(ft == 0), stop=(ft == 7))
                    o_t = small.tile([128, 1024], f32, tag="o_t", name="o_t", bufs=2)
                    nc.vector.tensor_tensor(o_t, pso, pbb, add)
                    nc.sync.dma_start(out=out_d.ap()[st * 128:(st + 1) * 128, :], in_=o_t)

            for _rep in range(repeat):
                body()

    nc.compile()
    return nc


def _prep_shared(qkv_w, qkv_b, proj_w, proj_b):
    f16 = np.float16
    wqT = np.ascontiguousarray(qkv_w[0:1024].T)          # [D, 1024]
    wkT = np.ascontiguousarray(qkv_w[1024:2048].T)
    wvT = np.ascontiguousarray(qkv_w[2048:3072].T)
    # wq_all[p, hp*8+dt, c] = wqT[dt*128+p, hp*128+c]
    wq = np.ascontiguousarray(
        wqT.reshape(8, 128, 8, 128).transpose(1, 2, 0, 3).reshape(128, 64, 128)).astype(f16)
    wk = np.ascontiguousarray(
        wkT.reshape(8, 128, 8, 128).transpose(1, 2, 0, 3).reshape(128, 64, 128)).astype(f16)
    # wv[p, dt, f] = wvT[dt*128+p, f] ; V' ones handled on-device by memset
    wv = np.ascontiguousarray(
        wvT.reshape(8, 128, 1024).transpose(1, 0, 2)).astype(f16)
    pw = np.ascontiguousarray(
        proj_w.T.reshape(8, 128, 1024).transpose(1, 0, 2)).astype(f16)
    bq = np.ascontiguousarray(qkv_b[0:1024].reshape(8, 128).T).astype(np.float32)
    bk = np.ascontiguousarray(qkv_b[1024:2048].reshape(8, 128).T).astype(np.float32)
    return dict(
        wq=wq, wk=wk, wv=wv, bq=bq, bk=bk,
        bv=np.ascontiguousarray(qkv_b[2048:3072]).astype(f16),
        pw=pw,
        pb=np.ascontiguousarray(proj_b).astype(np.float32),
    )


def _make_in_maps(x, qkv_w, qkv_b, proj_w, proj_b):
    x = np.asarray(x, np.float32)
    shared = _prep_shared(np.asarray(qkv_w, np.float32), np.asarray(qkv_b, np.float32),
                          np.asarray(proj_w, np.float32), np.asarray(proj_b, np.float32))
    in_maps = []
    for c in range(NCORES):
        b, half = c // 2, c % 2
        xT = np.ascontiguousarray(x[b].T).astype(np.float16)          # [D, S]
        m = dict(shared)
        m["xt"] = np.ascontiguousarray(xT.reshape(8, 128, S).transpose(1, 0, 2))
        m["xq"] = np.ascontiguousarray(
            xT[:, half * SQ:(half + 1) * SQ].reshape(8, 128, SQ).transpose(1, 0, 2))
        in_maps.append(m)
    return in_maps


def kernel(x, qkv_w, qkv_b, proj_w, proj_b):
    global LAST_EXEC_TIME_NS
    from concourse.bass_utils import run_bass_kernel_spmd

    in_maps = _make_in_maps(x, qkv_w, qkv_b, proj_w, proj_b)
    if "nc" not in _cache:
        _cache["nc"] = _build_nc()
    nc = _cache["nc"]

    res = run_bass_kernel_spmd(nc, in_maps, core_ids=list(range(NCORES)))
    LAST_EXEC_TIME_NS = res.exec_time_ns

    out = np.zeros((B, S, D), np.float32)
    for c in range(NCORES):
        b, half = c // 2, c % 2
        out[b, half * SQ:(half + 1) * SQ, :] = res.results[c]["out"]
    return out



# revision 4
# speedup vs baseline: 14.7154x; 14.7154x over previous
"""Multi-head self-attention on 8 TRN2 NeuronCores — v4 (hardware loops).

Same math/sharding as the v3 baseline ((batch x query-half) shards, fp16
compute, transposed-scores softmax with ones-column denominators), but
restructured around For_i hardware loops: on this stack each *unique*
instruction costs ~70us to dispatch while re-executions inside a hardware
loop are ~free, so the kernel is organized to minimize unique instructions
(~200 vs ~4300 unrolled). Matmul stationary operands cannot take dynamic
(register) offsets, so per-iteration stationary slices are first copied
into fixed staging tiles on the ACT engine.
"""

import numpy as np

B, S, D = 4, 2048, 1024
H, DK = 16, 64
SQ = S // 2
SCALE = 64 ** -0.5
NCORES = 8

_cache = {}
LAST_EXEC_TIME_NS = None


def _build_nc(repeat=1):
    import concourse.bass as bass
    import concourse.mybir as mybir
    import concourse.tile as tile
    from concourse import bacc
    from concourse.bass import ds, ts

    fp16 = mybir.dt.float16
    f32 = mybir.dt.float32
    mult = mybir.AluOpType.mult
    add = mybir.AluOpType.add

    nc = bacc.Bacc(target_bir_lowering=False, debug=False, num_devices=NCORES)

    # ---- DRAM parameters, already in SBUF layout ----
    xt_d = nc.dram_tensor("xt", [128, 8, S], fp16, kind="ExternalInput")
    xq_d = nc.dram_tensor("xq", [128, 8, SQ], fp16, kind="ExternalInput")
    wq_d = nc.dram_tensor("wq", [128, 64, 128], fp16, kind="ExternalInput")
    wk_d = nc.dram_tensor("wk", [128, 64, 128], fp16, kind="ExternalInput")
    wv_d = nc.dram_tensor("wv", [128, 8, 1024], fp16, kind="ExternalInput")
    bq_d = nc.dram_tensor("bq", [128, 8], f32, kind="ExternalInput")
    bk_d = nc.dram_tensor("bk", [128, 8], f32, kind="ExternalInput")
    bv_d = nc.dram_tensor("bv", [1024], fp16, kind="ExternalInput")
    pw_d = nc.dram_tensor("pw", [128, 8, 1024], fp16, kind="ExternalInput")
    pb_d = nc.dram_tensor("pb", [1024], f32, kind="ExternalInput")
    out_d = nc.dram_tensor("out", [SQ, D], f32, kind="ExternalOutput")

    def bcast_rows(ap, parts):
        return bass.AP(tensor=ap.tensor, offset=ap.offset, ap=[[0, parts], *ap.ap])

    with tile.TileContext(nc) as tc:
        with (
            tc.tile_pool(name="const", bufs=1) as const,
            tc.tile_pool(name="acts", bufs=1) as acts,
            tc.tile_pool(name="work", bufs=1) as work,
            tc.tile_pool(name="small", bufs=2) as small,
            tc.tile_pool(name="ps", bufs=2, space="PSUM") as ps,
            tc.tile_pool(name="psO", bufs=2, space="PSUM") as psO,
        ):
            bvb = const.tile([128, 1024], fp16, tag="bvb")
            nc.sync.dma_start(out=bvb, in_=bcast_rows(bv_d.ap(), 128))
            pbb = const.tile([128, 1024], f32, tag="pbb")
            nc.sync.dma_start(out=pbb, in_=bcast_rows(pb_d.ap(), 128))
            wq_all = const.tile([128, 64, 128], fp16, tag="wq_all")
            nc.sync.dma_start(out=wq_all, in_=wq_d.ap())
            wk_all = const.tile([128, 64, 128], fp16, tag="wk_all")
            nc.sync.dma_start(out=wk_all, in_=wk_d.ap())
            bq_all = const.tile([128, 8], f32, tag="bq_all")
            nc.sync.dma_start(out=bq_all, in_=bq_d.ap())
            bk_all = const.tile([128, 8], f32, tag="bk_all")
            nc.sync.dma_start(out=bk_all, in_=bk_d.ap())
            wv = const.tile([128, 8, 1024], fp16, tag="wv")
            nc.sync.dma_start(out=wv, in_=wv_d.ap())
            pw = const.tile([128, 8, 1024], fp16, tag="pw")
            nc.sync.dma_start(out=pw, in_=pw_d.ap())

            def body():
                xt = work.tile([128, 8, S], fp16, tag="xt", name="xt")
                nc.sync.dma_start(out=xt, in_=xt_d.ap())
                xq = work.tile([128, 8, SQ], fp16, tag="xq", name="xq")
                nc.sync.dma_start(out=xq, in_=xq_d.ap())

                # V' [seq-part, st, head, 65] with per-head ones column
                vt_all = acts.tile([128, 16, 16, 65], fp16, tag="vt", name="vt_all")
                nc.vector.memset(vt_all[:, :, :, 64], 1.0)

                # ---- stage A: V' projection, hw loop over 16 seq blocks ----
                xstage = work.tile([128, 8, 128], fp16, tag="xstage", name="xstage")
                with tc.For_i(0, 16) as st:
                    nc.scalar.copy(xstage, xt[:, :, ts(st, 128)])
                    psa = ps.tile([128, 1024], f32, tag="ps", name="psa")
                    for dt in range(8):
                        for o in (0, 512):
                            nc.tensor.matmul(psa[:, o:o + 512],
                                             xstage[:, dt, :],
                                             wv[:, dt, o:o + 512],
                                             start=(dt == 0), stop=(dt == 7))
                    nc.vector.tensor_tensor(
                        vt_all[:, ds(st, 1), :, 0:64],
                        psa.rearrange("p (a b) -> p a b", a=16),
                        bvb.rearrange("p (a b) -> p a b", a=16), add)

                otn_all = acts.tile([128, 8, SQ], fp16, tag="otn", name="otn_all")

                # ---- stage BC: per head-pair QK projection + attention ----
                wqstage = work.tile([128, 8, 128], fp16, tag="wqstage", name="wqstage")
                wkstage = work.tile([128, 8, 128], fp16, tag="wkstage", name="wkstage")
                vhp = work.tile([128, 16, 2, 65], fp16, tag="vhp", name="vhp")
                qt = work.tile([128, SQ], fp16, tag="qt", name="qt")
                kt_t = work.tile([128, S], fp16, tag="kt", name="kt_t")
                kstage = work.tile([128, 128], fp16, tag="kstage", name="kstage")
                vstage = work.tile([128, 2, 65], fp16, tag="vstage", name="vstage")
                e2 = [work.tile([128, SQ], fp16, tag=f"e{hh}", name=f"e{hh}")
                      for hh in range(2)]
                rec = small.tile([1, SQ], f32, tag="rec", name="rec", bufs=1)
                recb = small.tile([64, SQ], f32, tag="recb", name="recb", bufs=1)
                # dynamic-offset writes miscompile on partial partition
                # ranges, so compose both head halves statically then copy
                otn_stage = work.tile([128, SQ], fp16, tag="otn_stage",
                                      name="otn_stage")

                with tc.For_i(0, 8) as hp:
                    nc.scalar.copy(wqstage, wq_all[:, ts(hp, 8), :])
                    nc.scalar.copy(wkstage, wk_all[:, ts(hp, 8), :])
                    nc.scalar.copy(vhp, vt_all[:, :, ts(hp, 2), :])

                    psq = ps.tile([128, SQ], f32, tag="ps", name="psq")
                    for dt in range(8):
                        for o in (0, 512):
                            nc.tensor.matmul(psq[:, o:o + 512],
                                             wqstage[:, dt, :],
                                             xq[:, dt, o:o + 512],
                                             start=(dt == 0), stop=(dt == 7))
                    nc.vector.tensor_scalar(qt[:], psq, bq_all[:, ds(hp, 1)],
                                            None, add)

                    psk2 = [ps.tile([128, SQ], f32, tag="ps", name=f"psk{h}")
                            for h in range(2)]
                    for dt in range(8):
                        for half in range(2):
                            for o in (0, 512):
                                nc.tensor.matmul(
                                    psk2[half][:, o:o + 512],
                                    wkstage[:, dt, :],
                                    xt[:, dt, half * SQ + o:half * SQ + o + 512],
                                    start=(dt == 0), stop=(dt == 7))
                    for half in range(2):
                        nc.vector.tensor_scalar(kt_t[:, half * SQ:(half + 1) * SQ],
                                                psk2[half], bk_all[:, ds(hp, 1)],
                                                None, add)

                    sc2 = [ps.tile([128, SQ], f32, tag="ps", name=f"sc{hh}")
                           for hh in range(2)]
                    ot2 = [psO.tile([65, SQ], f32, tag="ot", name=f"ot{hh}")
                           for hh in range(2)]

                    def kt_body(kt_i, av_start, av_stop, dyn):
                        if dyn:
                            nc.scalar.copy(kstage, kt_t[:, ts(kt_i, 128)])
                            nc.scalar.copy(vstage, vhp[:, ds(kt_i, 1), :, :])
                        else:
                            nc.scalar.copy(
                                kstage, kt_t[:, kt_i * 128:(kt_i + 1) * 128])
                            nc.scalar.copy(vstage, vhp[:, kt_i, :, :])
                        for hh in range(2):
                            hsl = slice(hh * 64, (hh + 1) * 64)
                            for o in (0, 512):
                                nc.tensor.matmul(sc2[hh][:, o:o + 512],
                                                 kstage[hsl, :],
                                                 qt[hsl, o:o + 512],
                                                 start=True, stop=True)
                        for hh in range(2):
                            nc.scalar.activation(e2[hh][:], sc2[hh][:],
                                                 mybir.ActivationFunctionType.Exp,
                                                 scale=float(SCALE))
                            for o in (0, 512):
                                nc.tensor.matmul(ot2[hh][:, o:o + 512],
                                                 vstage[:, hh, :],
                                                 e2[hh][:, o:o + 512],
                                                 start=av_start, stop=av_stop)

                    kt_body(0, True, False, dyn=False)
                    with tc.For_i(1, 15) as kt:
                        kt_body(kt, False, False, dyn=True)
                    kt_body(15, False, True, dyn=False)

                    for hh in range(2):
                        nc.vector.reciprocal(rec, ot2[hh][64:65, :])
                        nc.gpsimd.partition_broadcast(recb, rec)
                        nc.vector.tensor_tensor(
                            otn_stage[hh * 64:(hh + 1) * 64, :],
                            ot2[hh][0:64, :], recb, mult)
                    nc.scalar.copy(otn_all[:, ds(hp, 1), :], otn_stage)

                # ---- stage D: output projection, hw loop over 8 q blocks ----
                ostage = work.tile([128, 8, 128], fp16, tag="ostage", name="ostage")
                with tc.For_i(0, 8) as qb:
                    nc.scalar.copy(ostage, otn_all[:, :, ts(qb, 128)])
                    pso = ps.tile([128, 1024], f32, tag="ps", name="pso")
                    for ft in range(8):
                        for o in (0, 512):
                            nc.tensor.matmul(pso[:, o:o + 512],
                                             ostage[:, ft, :],
                                             pw[:, ft, o:o + 512],
                                             start=(ft == 0), stop=(ft == 7))
                    o_t = small.tile([128, 1024], f32, tag="o_t", name="o_t")
                    nc.vector.tensor_tensor(o_t, pso, pbb, add)
                    nc.sync.dma_start(out=out_d.ap()[ts(qb, 128), :], in_=o_t)

            for _rep in range(repeat):
                body()

    nc.compile()
    return nc


def _prep_shared(qkv_w, qkv_b, proj_w, proj_b):
    f16 = np.float16
    wqT = np.ascontiguousarray(qkv_w[0:1024].T)          # [D, 1024]
    wkT = np.ascontiguousarray(qkv_w[1024:2048].T)
    wvT = np.ascontiguousarray(qkv_w[2048:3072].T)
    # wq_all[p, hp*8+dt, c] = wqT[dt*128+p, hp*128+c]
    wq = np.ascontiguousarray(
        wqT.reshape(8, 128, 8, 128).transpose(1, 2, 0, 3).reshape(128, 64, 128)).astype(f16)
    wk = np.ascontiguousarray(
        wkT.reshape(8, 128, 8, 128).transpose(1, 2, 0, 3).reshape(128, 64, 128)).astype(f16)
    # wv[p, dt, f] = wvT[dt*128+p, f] ; V' ones handled on-device by memset
    wv = np.ascontiguousarray(
        wvT.reshape(8, 128, 1024).transpose(1, 0, 2)).astype(f16)
    pw = np.ascontiguousarray(
        proj_w.T.reshape(8, 128, 1024).transpose(1, 0, 2)).astype(f16)
    bq = np.ascontiguousarray(qkv_b[0:1024].reshape(8, 128).T).astype(np.float32)
    bk = np.ascontiguousarray(qkv_b[1024:2048].reshape(8, 128).T).astype(np.float32)
    return dict(
        wq=wq, wk=wk, wv=wv, bq=bq, bk=bk,
        bv=np.ascontiguousarray(qkv_b[2048:3072]).astype(f16),
        pw=pw,
        pb=np.ascontiguousarray(proj_b).astype(np.float32),
    )


def _make_in_maps(x, qkv_w, qkv_b, proj_w, proj_b):
    x = np.asarray(x, np.float32)
    shared = _prep_shared(np.asarray(qkv_w, np.float32), np.asarray(qkv_b, np.float32),
                          np.asarray(proj_w, np.float32), np.asarray(proj_b, np.float32))
    in_maps = []
    for c in range(NCORES):
        b, half = c // 2, c % 2
        xT = np.ascontiguousarray(x[b].T).astype(np.float16)          # [D, S]
        m = dict(shared)
        m["xt"] = np.ascontiguousarray(xT.reshape(8, 128, S).transpose(1, 0, 2))
        m["xq"] = np.ascontiguousarray(
            xT[:, half * SQ:(half + 1) * SQ].reshape(8, 128, SQ).transpose(1, 0, 2))
        in_maps.append(m)
    return in_maps


def kernel(x, qkv_w, qkv_b, proj_w, proj_b):
    global LAST_EXEC_TIME_NS
    from concourse.bass_utils import run_bass_kernel_spmd

    in_maps = _make_in_maps(x, qkv_w, qkv_b, proj_w, proj_b)
    if "nc" not in _cache:
        _cache["nc"] = _build_nc()
    nc = _cache["nc"]

    res = run_bass_kernel_spmd(nc, in_maps, core_ids=list(range(NCORES)))
    LAST_EXEC_TIME_NS = res.exec_time_ns

    out = np.zeros((B, S, D), np.float32)
    for c in range(NCORES):
        b, half = c // 2, c % 2
        out[b, half * SQ:(half + 1) * SQ, :] = res.results[c]["out"]
    return out


# revision 5
# speedup vs baseline: 77.1231x; 5.2410x over previous
"""Multi-head self-attention on 8 TRN2 NeuronCores — v5 (nested hw loops).

Same math/sharding as v3/v4 ((batch x query-half) shards, fp16 compute,
transposed-scores softmax with ones-column denominators). On this stack
each *unique* instruction costs ~50-70us to dispatch while hardware-loop
re-executions are ~free, so every loop level is a For_i hardware loop:
seq blocks, head pairs, k blocks, and the contraction (dt/ft) loops.
Matmul stationaries cannot take dynamic offsets, so each iteration copies
its stationary slice into a fixed staging tile (ACT engine); PSUM
accumulation across loop iterations uses memset-zeroed banks with
start=False matmuls instead of peeled start/stop flags.
"""

import numpy as np

B, S, D = 4, 2048, 1024
H, DK = 16, 64
SQ = S // 2
SCALE = 64 ** -0.5
NCORES = 8

_cache = {}
LAST_EXEC_TIME_NS = None


def _build_nc(repeat=1):
    import concourse.bass as bass
    import concourse.mybir as mybir
    import concourse.tile as tile
    from concourse import bacc
    from concourse.bass import ds, ts

    fp16 = mybir.dt.float16
    f32 = mybir.dt.float32
    mult = mybir.AluOpType.mult
    add = mybir.AluOpType.add

    nc = bacc.Bacc(target_bir_lowering=False, debug=False, num_devices=NCORES)

    xt_d = nc.dram_tensor("xt", [128, 8, S], fp16, kind="ExternalInput")
    xq_d = nc.dram_tensor("xq", [128, 8, SQ], fp16, kind="ExternalInput")
    wq_d = nc.dram_tensor("wq", [128, 64, 128], fp16, kind="ExternalInput")
    wk_d = nc.dram_tensor("wk", [128, 64, 128], fp16, kind="ExternalInput")
    wv_d = nc.dram_tensor("wv", [128, 8, 1024], fp16, kind="ExternalInput")
    bq_d = nc.dram_tensor("bq", [128, 8], f32, kind="ExternalInput")
    bk_d = nc.dram_tensor("bk", [128, 8], f32, kind="ExternalInput")
    bv_d = nc.dram_tensor("bv", [1024], fp16, kind="ExternalInput")
    pw_d = nc.dram_tensor("pw", [128, 8, 1024], fp16, kind="ExternalInput")
    pb_d = nc.dram_tensor("pb", [1024], f32, kind="ExternalInput")
    out_d = nc.dram_tensor("out", [SQ, D], f32, kind="ExternalOutput")

    def bcast_rows(ap, parts):
        return bass.AP(tensor=ap.tensor, offset=ap.offset, ap=[[0, parts], *ap.ap])

    with tile.TileContext(nc) as tc:
        with (
            tc.tile_pool(name="const", bufs=1) as const,
            tc.tile_pool(name="acts", bufs=1) as acts,
            tc.tile_pool(name="work", bufs=1) as work,
            tc.tile_pool(name="small", bufs=2) as small,
            tc.tile_pool(name="ps", bufs=2, space="PSUM") as ps,
            tc.tile_pool(name="psO", bufs=2, space="PSUM") as psO,
        ):
            bvb = const.tile([128, 1024], fp16, tag="bvb")
            nc.sync.dma_start(out=bvb, in_=bcast_rows(bv_d.ap(), 128))
            pbb = const.tile([128, 1024], f32, tag="pbb")
            nc.sync.dma_start(out=pbb, in_=bcast_rows(pb_d.ap(), 128))
            wq_all = const.tile([128, 64, 128], fp16, tag="wq_all")
            nc.sync.dma_start(out=wq_all, in_=wq_d.ap())
            wk_all = const.tile([128, 64, 128], fp16, tag="wk_all")
            nc.sync.dma_start(out=wk_all, in_=wk_d.ap())
            bq_all = const.tile([128, 8], f32, tag="bq_all")
            nc.sync.dma_start(out=bq_all, in_=bq_d.ap())
            bk_all = const.tile([128, 8], f32, tag="bk_all")
            nc.sync.dma_start(out=bk_all, in_=bk_d.ap())
            wv = const.tile([128, 8, 1024], fp16, tag="wv")
            nc.sync.dma_start(out=wv, in_=wv_d.ap())
            pw = const.tile([128, 8, 1024], fp16, tag="pw")
            nc.sync.dma_start(out=pw, in_=pw_d.ap())

            def body():
                xt = work.tile([128, 8, S], fp16, tag="xt", name="xt")
                nc.sync.dma_start(out=xt, in_=xt_d.ap())
                xq = work.tile([128, 8, SQ], fp16, tag="xq", name="xq")
                nc.sync.dma_start(out=xq, in_=xq_d.ap())

                vt_all = acts.tile([128, 16, 16, 65], fp16, tag="vt", name="vt_all")
                nc.vector.memset(vt_all[:, :, :, 64], 1.0)

                # ---- stage A: V' projection ----
                xstage = work.tile([128, 8, 128], fp16, tag="xstage", name="xstage")
                x_dt = work.tile([128, 128], fp16, tag="x_dt", name="x_dt")
                with tc.For_i(0, 16) as st:
                    nc.scalar.copy(xstage, xt[:, :, ts(st, 128)])
                    psa = ps.tile([128, 1024], f32, tag="ps", name="psa")
                    nc.vector.memset(psa, 0.0)
                    with tc.For_i(0, 8) as dt:
                        nc.scalar.copy(x_dt, xstage[:, ds(dt, 1), :])
                        for o in (0, 512):
                            nc.tensor.matmul(psa[:, o:o + 512],
                                             x_dt,
                                             wv[:, ds(dt, 1), o:o + 512],
                                             start=False, stop=False,
                                             skip_group_check=True)
                    nc.vector.tensor_tensor(
                        vt_all[:, ds(st, 1), :, 0:64],
                        psa.rearrange("p (a b) -> p a b", a=16),
                        bvb.rearrange("p (a b) -> p a b", a=16), add)

                otn_all = acts.tile([128, 8, SQ], fp16, tag="otn", name="otn_all")

                # ---- stage BC: per head-pair QK projection + attention ----
                w_dt = work.tile([128, 128], fp16, tag="w_dt", name="w_dt")
                w_dt2 = work.tile([128, 128], fp16, tag="w_dt2", name="w_dt2")
                vhp = work.tile([128, 16, 2, 65], fp16, tag="vhp", name="vhp")
                qt = work.tile([128, SQ], fp16, tag="qt", name="qt")
                kt_t = work.tile([128, S], fp16, tag="kt", name="kt_t")
                kstage = work.tile([128, 128], fp16, tag="kstage", name="kstage")
                vstage = work.tile([128, 2, 65], fp16, tag="vstage", name="vstage")
                e2 = [work.tile([128, SQ], fp16, tag=f"e{hh}", name=f"e{hh}")
                      for hh in range(2)]
                rec = small.tile([1, SQ], f32, tag="rec", name="rec", bufs=1)
                recb = small.tile([64, SQ], f32, tag="recb", name="recb", bufs=1)
                # dynamic-offset writes miscompile on partial partition
                # ranges, so compose both head halves statically then copy
                otn_stage = work.tile([128, SQ], fp16, tag="otn_stage",
                                      name="otn_stage")

                with tc.For_i(0, 8) as hp:
                    nc.scalar.copy(vhp, vt_all[:, :, ts(hp, 2), :])

                    psq = ps.tile([128, SQ], f32, tag="ps", name="psq")
                    psk0 = ps.tile([128, SQ], f32, tag="ps", name="psk0")
                    psk1 = psO.tile([128, SQ], f32, tag="ot", name="psk1")
                    nc.vector.memset(psq, 0.0)
                    nc.vector.memset(psk0, 0.0)
                    nc.vector.memset(psk1, 0.0)
                    psk2 = [psk0, psk1]
                    with tc.For_i(0, 8) as dt:
                        nc.scalar.copy(w_dt, wq_all[:, ds(hp * 8 + dt, 1), :])
                        nc.scalar.copy(w_dt2, wk_all[:, ds(hp * 8 + dt, 1), :])
                        for o in (0, 512):
                            nc.tensor.matmul(psq[:, o:o + 512],
                                             w_dt,
                                             xq[:, ds(dt, 1), o:o + 512],
                                             start=False, stop=False,
                                             skip_group_check=True)
                        for half in range(2):
                            for o in (0, 512):
                                nc.tensor.matmul(
                                    psk2[half][:, o:o + 512],
                                    w_dt2,
                                    xt[:, ds(dt, 1), half * SQ + o:half * SQ + o + 512],
                                    start=False, stop=False,
                                    skip_group_check=True)
                    nc.vector.tensor_scalar(qt[:], psq, bq_all[:, ds(hp, 1)],
                                            None, add)
                    for half in range(2):
                        nc.vector.tensor_scalar(kt_t[:, half * SQ:(half + 1) * SQ],
                                                psk2[half], bk_all[:, ds(hp, 1)],
                                                None, add)

                    sc2 = [ps.tile([128, SQ], f32, tag="ps", name=f"sc{hh}")
                           for hh in range(2)]
                    ot2 = [psO.tile([128, SQ], f32, tag="ot", name=f"ot{hh}")
                           for hh in range(2)]
                    nc.vector.memset(ot2[0][0:65, :], 0.0)
                    nc.vector.memset(ot2[1][0:65, :], 0.0)

                    with tc.For_i(0, 16) as kt:
                        nc.scalar.copy(kstage, kt_t[:, ts(kt, 128)])
                        nc.scalar.copy(vstage, vhp[:, ds(kt, 1), :, :])
                        for hh in range(2):
                            hsl = slice(hh * 64, (hh + 1) * 64)
                            for o in (0, 512):
                                nc.tensor.matmul(sc2[hh][:, o:o + 512],
                                                 kstage[hsl, :],
                                                 qt[hsl, o:o + 512],
                                                 start=True, stop=True)
                        for hh in range(2):
                            nc.scalar.activation(e2[hh][:], sc2[hh][:],
                                                 mybir.ActivationFunctionType.Exp,
                                                 scale=float(SCALE))
                            for o in (0, 512):
                                nc.tensor.matmul(ot2[hh][0:65, o:o + 512],
                                                 vstage[:, hh, :],
                                                 e2[hh][:, o:o + 512],
                                                 start=False, stop=False,
                                                 skip_group_check=True)

                    for hh in range(2):
                        nc.vector.reciprocal(rec, ot2[hh][64:65, :])
                        nc.gpsimd.partition_broadcast(recb, rec)
                        nc.vector.tensor_tensor(
                            otn_stage[hh * 64:(hh + 1) * 64, :],
                            ot2[hh][0:64, :], recb, mult)
                    nc.scalar.copy(otn_all[:, ds(hp, 1), :], otn_stage)

                # ---- stage D: output projection ----
                ostage = work.tile([128, 8, 128], fp16, tag="ostage", name="ostage")
                o_dt = work.tile([128, 128], fp16, tag="o_dt", name="o_dt")
                with tc.For_i(0, 8) as qb:
                    nc.scalar.copy(ostage, otn_all[:, :, ts(qb, 128)])
                    pso = ps.tile([128, 1024], f32, tag="ps", name="pso")
                    nc.vector.memset(pso, 0.0)
                    with tc.For_i(0, 8) as ft:
                        nc.scalar.copy(o_dt, ostage[:, ds(ft, 1), :])
                        for o in (0, 512):
                            nc.tensor.matmul(pso[:, o:o + 512],
                                             o_dt,
                                             pw[:, ds(ft, 1), o:o + 512],
                                             start=False, stop=False,
                                             skip_group_check=True)
                    o_t = small.tile([128, 1024], f32, tag="o_t", name="o_t")
                    nc.vector.tensor_tensor(o_t, pso, pbb, add)
                    nc.sync.dma_start(out=out_d.ap()[ts(qb, 128), :], in_=o_t)

            for _rep in range(repeat):
                body()

    nc.compile()
    return nc


def _prep_shared(qkv_w, qkv_b, proj_w, proj_b):
    f16 = np.float16
    wqT = np.ascontiguousarray(qkv_w[0:1024].T)          # [D, 1024]
    wkT = np.ascontiguousarray(qkv_w[1024:2048].T)
    wvT = np.ascontiguousarray(qkv_w[2048:3072].T)
    # wq_all[p, hp*8+dt, c] = wqT[dt*128+p, hp*128+c]
    wq = np.ascontiguousarray(
        wqT.reshape(8, 128, 8, 128).transpose(1, 2, 0, 3).reshape(128, 64, 128)).astype(f16)
    wk = np.ascontiguousarray(
        wkT.reshape(8, 128, 8, 128).transpose(1, 2, 0, 3).reshape(128, 64, 128)).astype(f16)
    # wv[p, dt, f] = wvT[dt*128+p, f] ; V' ones handled on-device by memset
    wv = np.ascontiguousarray(
        wvT.reshape(8, 128, 1024).transpose(1, 0, 2)).astype(f16)
    pw = np.ascontiguousarray(
        proj_w.T.reshape(8, 128, 1024).transpose(1, 0, 2)).astype(f16)
    bq = np.ascontiguousarray(qkv_b[0:1024].reshape(8, 128).T).astype(np.float32)
    bk = np.ascontiguousarray(qkv_b[1024:2048].reshape(8, 128).T).astype(np.float32)
    return dict(
        wq=wq, wk=wk, wv=wv, bq=bq, bk=bk,
        bv=np.ascontiguousarray(qkv_b[2048:3072]).astype(f16),
        pw=pw,
        pb=np.ascontiguousarray(proj_b).astype(np.float32),
    )


def _make_in_maps(x, qkv_w, qkv_b, proj_w, proj_b):
    x = np.asarray(x, np.float32)
    shared = _prep_shared(np.asarray(qkv_w, np.float32), np.asarray(qkv_b, np.float32),
                          np.asarray(proj_w, np.float32), np.asarray(proj_b, np.float32))
    in_maps = []
    for c in range(NCORES):
        b, half = c // 2, c % 2
        xT = np.ascontiguousarray(x[b].T).astype(np.float16)          # [D, S]
        m = dict(shared)
        m["xt"] = np.ascontiguousarray(xT.reshape(8, 128, S).transpose(1, 0, 2))
        m["xq"] = np.ascontiguousarray(
            xT[:, half * SQ:(half + 1) * SQ].reshape(8, 128, SQ).transpose(1, 0, 2))
        in_maps.append(m)
    return in_maps


def kernel(x, qkv_w, qkv_b, proj_w, proj_b):
    global LAST_EXEC_TIME_NS
    from concourse.bass_utils import run_bass_kernel_spmd

    in_maps = _make_in_maps(x, qkv_w, qkv_b, proj_w, proj_b)
    if "nc" not in _cache:
        _cache["nc"] = _build_nc()
    nc = _cache["nc"]

    res = run_bass_kernel_spmd(nc, in_maps, core_ids=list(range(NCORES)))
    LAST_EXEC_TIME_NS = res.exec_time_ns

    out = np.zeros((B, S, D), np.float32)
    for c in range(NCORES):
        b, half = c // 2, c % 2
        out[b, half * SQ:(half + 1) * SQ, :] = res.results[c]["out"]
    return out


# revision 9
# speedup vs baseline: 146.8552x; 1.9042x over previous
"""Multi-head self-attention on 8 TRN2 NeuronCores — v6 (3 hardware loops).

Same math/sharding as v3-v5 ((batch x query-half) shards, fp16 compute,
transposed-scores softmax with ones-column denominators). On this stack
total device time ~= (unique instructions) x ~50us, where each For_i
hardware loop adds ~100 control instructions across the 5 engine queues
but makes re-executions ~free. v6 therefore uses exactly three loops:
seq-blocks (V' projection), head-pairs (QK projection + attention +
a folded output-projection partial), and the k-block loop nested in the
head-pair loop. The output projection accumulates per-head-pair partials
into an SBUF accumulator (out = sum_hp otn_hp^T @ pw_hp), which removes
the separate projection stage. Matmul stationaries cannot take dynamic
offsets, so loop-variant stationaries are staged into fixed tiles; PSUM
accumulation across k-block iterations uses memset-zeroed banks with
start=False matmuls.
"""

import numpy as np

B, S, D = 4, 2048, 1024
H, DK = 16, 64
SQ = S // 2
SCALE = 64 ** -0.5
NCORES = 8

_cache = {}
LAST_EXEC_TIME_NS = None


def _build_nc(repeat=1):
    import concourse.bass as bass
    import concourse.mybir as mybir
    import concourse.tile as tile
    from concourse import bacc
    from concourse.bass import ds, ts

    fp16 = mybir.dt.float16
    f32 = mybir.dt.float32
    mult = mybir.AluOpType.mult
    add = mybir.AluOpType.add

    nc = bacc.Bacc(target_bir_lowering=False, debug=False, num_devices=NCORES)

    xt_d = nc.dram_tensor("xt", [128, 8, S], fp16, kind="ExternalInput")
    xq_d = nc.dram_tensor("xq", [128, 8, SQ], fp16, kind="ExternalInput")
    wq_d = nc.dram_tensor("wq", [128, 64, 128], fp16, kind="ExternalInput")
    wk_d = nc.dram_tensor("wk", [128, 64, 128], fp16, kind="ExternalInput")
    wv_d = nc.dram_tensor("wv", [128, 8, 1024], fp16, kind="ExternalInput")
    bq_d = nc.dram_tensor("bq", [128, 8], f32, kind="ExternalInput")
    bk_d = nc.dram_tensor("bk", [128, 8], f32, kind="ExternalInput")
    bv_d = nc.dram_tensor("bv", [1024], fp16, kind="ExternalInput")
    pw_d = nc.dram_tensor("pw", [128, 8, 1024], fp16, kind="ExternalInput")
    pb_d = nc.dram_tensor("pb", [1024], f32, kind="ExternalInput")
    out_d = nc.dram_tensor("out", [SQ, D], f32, kind="ExternalOutput")

    def bcast_rows(ap, parts):
        return bass.AP(tensor=ap.tensor, offset=ap.offset, ap=[[0, parts], *ap.ap])

    with tile.TileContext(nc) as tc:
        with (
            tc.tile_pool(name="const", bufs=1) as const,
            tc.tile_pool(name="acts", bufs=1) as acts,
            tc.tile_pool(name="work", bufs=1) as work,
            tc.tile_pool(name="small", bufs=2) as small,
            tc.tile_pool(name="ps", bufs=2, space="PSUM") as ps,
            tc.tile_pool(name="psO", bufs=2, space="PSUM") as psO,
        ):
            bvb = const.tile([128, 1024], fp16, tag="bvb")
            nc.sync.dma_start(out=bvb, in_=bcast_rows(bv_d.ap(), 128))
            wq_all = const.tile([128, 64, 128], fp16, tag="wq_all")
            nc.sync.dma_start(out=wq_all, in_=wq_d.ap())
            wk_all = const.tile([128, 64, 128], fp16, tag="wk_all")
            nc.sync.dma_start(out=wk_all, in_=wk_d.ap())
            bq_all = const.tile([128, 8], f32, tag="bq_all")
            nc.sync.dma_start(out=bq_all, in_=bq_d.ap())
            bk_all = const.tile([128, 8], f32, tag="bk_all")
            nc.sync.dma_start(out=bk_all, in_=bk_d.ap())
            wv = const.tile([128, 8, 1024], fp16, tag="wv")
            nc.sync.dma_start(out=wv, in_=wv_d.ap())
            pw = const.tile([128, 8, 1024], fp16, tag="pw")
            nc.sync.dma_start(out=pw, in_=pw_d.ap())

            def body():
                xt = work.tile([128, 8, S], fp16, tag="xt", name="xt")
                nc.sync.dma_start(out=xt, in_=xt_d.ap())
                xq = work.tile([128, 8, SQ], fp16, tag="xq", name="xq")
                nc.sync.dma_start(out=xq, in_=xq_d.ap())
                # out accumulator [q-part, q-block, d], initialized with bias
                acc = acts.tile([128, 8, 1024], f32, tag="acc", name="acc")
                nc.sync.dma_start(
                    out=acc, in_=bcast_rows(bcast_rows(pb_d.ap(), 8), 128))

                vt_all = acts.tile([128, 16, 16, 65], fp16, tag="vt", name="vt_all")
                nc.vector.memset(vt_all[:, :, :, 64], 1.0)

                # ---- stage A: V' projection, hw loop over 16 seq blocks ----
                xstage = work.tile([128, 8, 128], fp16, tag="xstage", name="xstage")
                with tc.For_i(0, 16) as st:
                    nc.scalar.copy(xstage, xt[:, :, ts(st, 128)])
                    psa = ps.tile([128, 1024], f32, tag="ps", name="psa")
                    for dt in range(8):
                        for o in (0, 512):
                            nc.tensor.matmul(psa[:, o:o + 512],
                                             xstage[:, dt, :],
                                             wv[:, dt, o:o + 512],
                                             start=(dt == 0), stop=(dt == 7))
                    nc.vector.tensor_tensor(
                        vt_all[:, ds(st, 1), :, 0:64],
                        psa.rearrange("p (a b) -> p a b", a=16),
                        bvb.rearrange("p (a b) -> p a b", a=16), add)

                # ---- stage BC: per head-pair QK proj + attention + out fold ----
                wqstage = work.tile([128, 8, 128], fp16, tag="wqstage", name="wqstage")
                wkstage = work.tile([128, 8, 128], fp16, tag="wkstage", name="wkstage")
                vhp = work.tile([128, 16, 2, 65], fp16, tag="vhp", name="vhp")
                qt = work.tile([128, SQ], fp16, tag="qt", name="qt")
                kt_t = work.tile([128, S], fp16, tag="kt", name="kt_t")
                kstage = work.tile([128, 128], fp16, tag="kstage", name="kstage")
                vstage = work.tile([128, 2, 65], fp16, tag="vstage", name="vstage")
                e1 = work.tile([128, SQ], fp16, tag="e1", name="e1")
                e2 = [e1, e1]
                rec = small.tile([1, SQ], fp16, tag="rec", name="rec", bufs=1)
                recb = small.tile([64, SQ], fp16, tag="recb", name="recb", bufs=1)
                otn_stage = work.tile([128, SQ], fp16, tag="otn_stage",
                                      name="otn_stage")

                with tc.For_i(0, 8) as hp:
                    nc.scalar.copy(wqstage, wq_all[:, ts(hp, 8), :])
                    nc.scalar.copy(wkstage, wk_all[:, ts(hp, 8), :])
                    nc.scalar.copy(vhp, vt_all[:, :, ts(hp, 2), :])

                    psq = ps.tile([128, SQ], f32, tag="ps", name="psq")
                    for dt in range(8):
                        for o in (0, 512):
                            nc.tensor.matmul(psq[:, o:o + 512],
                                             wqstage[:, dt, :],
                                             xq[:, dt, o:o + 512],
                                             start=(dt == 0), stop=(dt == 7))
                    nc.vector.tensor_scalar(qt[:], psq, bq_all[:, ds(hp, 1)],
                                            None, add)

                    psk2 = [ps.tile([128, SQ], f32, tag="ps", name=f"psk{h}")
                            for h in range(2)]
                    for dt in range(8):
                        for half in range(2):
                            for o in (0, 512):
                                nc.tensor.matmul(
                                    psk2[half][:, o:o + 512],
                                    wkstage[:, dt, :],
                                    xt[:, dt, half * SQ + o:half * SQ + o + 512],
                                    start=(dt == 0), stop=(dt == 7))
                    for half in range(2):
                        nc.vector.tensor_scalar(kt_t[:, half * SQ:(half + 1) * SQ],
                                                psk2[half], bk_all[:, ds(hp, 1)],
                                                None, add)

                    sc2 = [ps.tile([128, SQ], f32, tag="ps", name=f"sc{hh}")
                           for hh in range(2)]
                    ot2 = [psO.tile([128, SQ], f32, tag="ot", name=f"ot{hh}")
                           for hh in range(2)]
                    nc.vector.memset(ot2[0][0:65, :], 0.0)
                    nc.vector.memset(ot2[1][0:65, :], 0.0)

                    with tc.For_i(0, 16) as kt:
                        nc.scalar.copy(kstage, kt_t[:, ts(kt, 128)])
                        nc.scalar.copy(vstage, vhp[:, ds(kt, 1), :, :])
                        for hh in range(2):
                            hsl = slice(hh * 64, (hh + 1) * 64)
                            for o in (0, 512):
                                nc.tensor.matmul(sc2[hh][:, o:o + 512],
                                                 kstage[hsl, :],
                                                 qt[hsl, o:o + 512],
                                                 start=True, stop=True)
                        for hh in range(2):
                            nc.scalar.activation(e2[hh][:], sc2[hh][:],
                                                 mybir.ActivationFunctionType.Exp,
                                                 scale=float(SCALE))
                            for o in (0, 512):
                                nc.tensor.matmul(ot2[hh][0:65, o:o + 512],
                                                 vstage[:, hh, :],
                                                 e2[hh][:, o:o + 512],
                                                 start=False, stop=False,
                                                 skip_group_check=True)

                    for hh in range(2):
                        with nc.allow_low_precision(reason="fp16 softmax denom"):
                            nc.vector.reciprocal(rec, ot2[hh][64:65, :])
                        nc.gpsimd.partition_broadcast(recb, rec)
                        nc.vector.tensor_tensor(
                            otn_stage[hh * 64:(hh + 1) * 64, :],
                            ot2[hh][0:64, :], recb, mult)

                    # folded output projection: acc[:, qb, :] += otn^T @ pw_hp
                    for qb in range(8):
                        pspr = ps.tile([128, 1024], f32, tag="ps",
                                       name=f"pspr{qb}")
                        for o in (0, 512):
                            nc.tensor.matmul(pspr[:, o:o + 512],
                                             otn_stage[:, qb * 128:(qb + 1) * 128],
                                             pw[:, ds(hp, 1), o:o + 512],
                                             start=True, stop=True)
                        nc.vector.tensor_tensor(acc[:, qb, :], acc[:, qb, :],
                                                pspr, add)

                for qb in range(8):
                    nc.sync.dma_start(out=out_d.ap()[qb * 128:(qb + 1) * 128, :],
                                      in_=acc[:, qb, :])

            for _rep in range(repeat):
                body()

    nc.compile()
    return nc


def _prep_shared(qkv_w, qkv_b, proj_w, proj_b):
    f16 = np.float16
    wqT = np.ascontiguousarray(qkv_w[0:1024].T)          # [D, 1024]
    wkT = np.ascontiguousarray(qkv_w[1024:2048].T)
    wvT = np.ascontiguousarray(qkv_w[2048:3072].T)
    # wq_all[p, hp*8+dt, c] = wqT[dt*128+p, hp*128+c]
    wq = np.ascontiguousarray(
        wqT.reshape(8, 128, 8, 128).transpose(1, 2, 0, 3).reshape(128, 64, 128)).astype(f16)
    wk = np.ascontiguousarray(
        wkT.reshape(8, 128, 8, 128).transpose(1, 2, 0, 3).reshape(128, 64, 128)).astype(f16)
    # wv[p, dt, f] = wvT[dt*128+p, f] ; V' ones handled on-device by memset
    wv = np.ascontiguousarray(
        wvT.reshape(8, 128, 1024).transpose(1, 0, 2)).astype(f16)
    pw = np.ascontiguousarray(
        proj_w.T.reshape(8, 128, 1024).transpose(1, 0, 2)).astype(f16)
    bq = np.ascontiguousarray(qkv_b[0:1024].reshape(8, 128).T).astype(np.float32)
    bk = np.ascontiguousarray(qkv_b[1024:2048].reshape(8, 128).T).astype(np.float32)
    return dict(
        wq=wq, wk=wk, wv=wv, bq=bq, bk=bk,
        bv=np.ascontiguousarray(qkv_b[2048:3072]).astype(f16),
        pw=pw,
        pb=np.ascontiguousarray(proj_b).astype(np.float32),
    )


def _make_in_maps(x, qkv_w, qkv_b, proj_w, proj_b):
    x = np.asarray(x, np.float32)
    shared = _prep_shared(np.asarray(qkv_w, np.float32), np.asarray(qkv_b, np.float32),
                          np.asarray(proj_w, np.float32), np.asarray(proj_b, np.float32))
    in_maps = []
    for c in range(NCORES):
        b, half = c // 2, c % 2
        xT = np.ascontiguousarray(x[b].T).astype(np.float16)          # [D, S]
        m = dict(shared)
        m["xt"] = np.ascontiguousarray(xT.reshape(8, 128, S).transpose(1, 0, 2))
        m["xq"] = np.ascontiguousarray(
            xT[:, half * SQ:(half + 1) * SQ].reshape(8, 128, SQ).transpose(1, 0, 2))
        in_maps.append(m)
    return in_maps


def kernel(x, qkv_w, qkv_b, proj_w, proj_b):
    global LAST_EXEC_TIME_NS
    from concourse.bass_utils import run_bass_kernel_spmd

    in_maps = _make_in_maps(x, qkv_w, qkv_b, proj_w, proj_b)
    if "nc" not in _cache:
        _cache["nc"] = _build_nc()
    nc = _cache["nc"]

    res = run_bass_kernel_spmd(nc, in_maps, core_ids=list(range(NCORES)))
    LAST_EXEC_TIME_NS = res.exec_time_ns

    out = np.zeros((B, S, D), np.float32)
    for c in range(NCORES):
        b, half = c // 2, c % 2
        out[b, half * SQ:(half + 1) * SQ, :] = res.results[c]["out"]
    return out
